# revision 1
# baseline (speedup 1.0000x reference)
"""CT-LSTM cell kernel for Trainium2, data-parallel over 8 NeuronCores.

Computes, for B=1048576 rows:
    z = [x, h_prev] @ W + b            (W = concat of 5 [80,16] mats -> [80,80])
    i, f, o, c~ = tanh(z[:, 0:64] split); decay = softplus(z[:, 64:80])
    c_next = f * (c_prev * exp(-decay*dt)) + i * c~
    h_next = o * tanh(c_next)

Strategy (~1.9x vs the fp32 baseline; ACT-engine-bound at ~34us per
32768-row mega-group per core):
  * All I/O and SBUF elementwise tensors are fp16 (halves DMA bytes, 1
    cycle/row matmuls, 2x/4x DVE modes); PSUM accumulates fp32; softplus
    via exp + ln(1+x) (AF.Softplus has no table on this stack).
  * PSUM: gate matmuls fill [128, 3, 512] groups (24 subtiles, 8 x 64
    cols per 2KB bank exactly -> one contiguous 1536-col tanh drain into
    a mega-resident fp16 gates buffer); decay cols fill a separate
    full bank per 32 subtiles (one DVE copy).  3*2 + 1*2 = all 8 banks.
  * The decay/cell chain runs at mega scope in 4 sub-slices so the
    serial ACT<->DVE ping-pong pipelines; exp(zd)/ln stay full-mega so
    the natural_log table window is one contiguous block (exp and
    tanh(c_next) share the other table set => exactly 2 table loads per
    mega-group).  Outputs pack into one [128, J, 32] tile whose flush is
    deferred one mega-group so output DMAs never wait.
  * DMA holds (the issuing sequencer is held for the WHOLE transfer
    incl. waits in the cost model) are split: x/h/cp/dt on SP in 4096-col
    slabs with a 4-deep prefetch ring (the single biggest win: the ACT
    pipeline never starves at group boundaries), outputs + weights on
    Pool (SWDGE).  ACT never issues DMAs - it is the bottleneck engine.
"""

import sys

import numpy as np

sys.path.insert(0, "/opt/trn_rl_repo")

from concourse import bacc, bass, mybir, tile  # noqa: E402
from concourse.bass_utils import run_bass_kernel_spmd  # noqa: E402

F32 = mybir.dt.float32
F16 = mybir.dt.float16
AF = mybir.ActivationFunctionType
ALU = mybir.AluOpType

N_CORES = 8
BATCH = 1048576
R = BATCH // N_CORES  # rows per core = 131072
D_X = 64
D_H = 16
KD = D_X + D_H + 1  # 81 contraction rows (incl. bias row)
import os as _os

N_SLICE = int(_os.environ.get("K_NSLICE", "4"))  # chain sub-slices per mega
ACT_SLICE = int(_os.environ.get("K_ACTSLICE", "4"))  # E/tanh sub-slices
DMACHUNK = int(_os.environ.get("K_DMACHUNK", "4096"))
GSUB = int(_os.environ.get("K_GSUB", "24"))  # subtiles per gate psum group
DSUB = 32  # subtiles per decay psum bank (32 x 16 cols = 2KB)


def build_program(rows, mega, chunk, n_cores=N_CORES):
    """Build + compile the Bass program (same NEFF for every core)."""
    assert rows % mega == 0 and mega % chunk == 0 and chunk == 2048
    n_mega = rows // mega
    J = mega // 128  # subtiles per mega-group
    JH = J * D_H
    n_chunk = mega // chunk  # chunks per mega-group
    spc = chunk // 128  # subtiles per chunk = 16
    dmachunk = min(DMACHUNK, mega)
    n_dma = mega // dmachunk
    cpd = dmachunk // chunk  # chunks per dma slab
    jcols = rows // 128
    assert J % N_SLICE == 0
    JS = J // N_SLICE  # subtiles per chain slice

    nc = bacc.Bacc(
        "TRN2",
        target_bir_lowering=False,
        debug=False,
        num_devices=n_cores,
    )
    xT = nc.dram_tensor("xT", [D_X, rows], F16, kind="ExternalInput").ap()
    hT = nc.dram_tensor("hT", [D_H + 1, rows], F16, kind="ExternalInput").ap()
    cp = nc.dram_tensor("cp", [128, jcols, D_H], F16, kind="ExternalInput").ap()
    dt = nc.dram_tensor("dt", [128, jcols], F16, kind="ExternalInput").ap()
    w64 = nc.dram_tensor("w64", [KD, 64], F16, kind="ExternalInput").ap()
    w16 = nc.dram_tensor("w16", [KD, D_H], F16, kind="ExternalInput").ap()
    # packed output: [..., 0:16] = h_next, [..., 16:32] = c_next
    hc = nc.dram_tensor("hc", [128, jcols, 2 * D_H], F16, kind="ExternalOutput").ap()

    with tile.TileContext(nc) as tc:
        with (
            tc.tile_pool(name="wbp", bufs=1) as wbp,
            tc.tile_pool(name="cmb", bufs=int(_os.environ.get("K_CMBBUFS", "4"))) as cmb_pool,
            tc.tile_pool(name="psG", bufs=(1 if GSUB == 48 else 2),
                         space="PSUM") as psG_pool,
            tc.tile_pool(name="psD", bufs=2, space="PSUM") as psD_pool,
            tc.tile_pool(name="gates", bufs=2) as gates_pool,
            tc.tile_pool(name="dtb", bufs=2) as dtb_pool,
            tc.tile_pool(name="zd", bufs=2) as zd_pool,
            tc.tile_pool(name="cpt", bufs=2) as cp_pool,
            tc.tile_pool(name="dtt", bufs=2) as dt_pool,
            tc.tile_pool(name="hcout", bufs=2) as hc_pool,
        ):
            # weights ride the Pool queue so SP can start the first x/h
            # slabs immediately
            w64_t = wbp.tile([KD, 64], F16)
            nc.gpsimd.dma_start(w64_t[:], w64[:, :])
            w16_t = wbp.tile([KD, D_H], F16)
            nc.gpsimd.dma_start(w16_t[:], w16[:, :])

            # Software-pipelined emission: phase A (DMA + GEMM + drains) of
            # group g is emitted BEFORE the decay chain of group g-1 so the
            # serial chain hides under dense work.
            state = {}

            def r3(ap2d, inner=D_H):
                return ap2d.rearrange("p (a b) -> p a b", b=inner)

            def phase_a(g):
                g0 = g * J
                zdb = zd_pool.tile([128, JH], F16, tag="zd", name=f"zd{g}")
                gates = gates_pool.tile([128, J * 64], F16, tag="gt",
                                        name=f"gt{g}")

                # PSUM: gate groups of 24 subtiles (3 banks, 8 x 64 cols
                # filling each bank exactly) + decay banks of 32 subtiles
                # (32 x 16 cols = one full bank); 3*2 + 1*2 = 8 banks.
                # Groups are decoupled from the DMA slab structure: each
                # matmul reads whichever cmbT slab holds its columns.
                # The very first slabs are smaller so the pipeline fills
                # sooner (first drain isn't gated on a full 4096-col load).
                if g == 0:
                    sizes = [2048, 2048] + [dmachunk] * ((mega - 4096) // dmachunk)
                else:
                    sizes = [dmachunk] * (mega // dmachunk)
                slabs = []
                roff = g * mega
                for sz in sizes:
                    slabs.append((roff, sz // 128))
                    roff += sz
                si = 0
                sub_in_slab = 0
                n_sub = 0
                cmbT = None
                psG = None
                psD = None
                g_start = 0
                g_len = 0
                for s in range(J):
                    if sub_in_slab == n_sub:
                        off, n_sub = slabs[si]
                        si += 1
                        sub_in_slab = 0
                        cmbT = cmb_pool.tile([KD, n_sub * 128], F16,
                                             name="cmbT")
                        nc.sync.dma_start(
                            cmbT[0:D_X, :], xT[:, off : off + n_sub * 128]
                        )
                        nc.sync.dma_start(
                            cmbT[D_X:KD, :], hT[:, off : off + n_sub * 128]
                        )
                    if psG is None:
                        g_start = s
                        g_len = min(GSUB, J - s)
                        psG = psG_pool.tile(
                            [128, GSUB // 8, 512], F32, name="psG")
                    if s % DSUB == 0:
                        psD = psD_pool.tile([128, 512], F32, name="psD")
                    ls = s - g_start
                    col = sub_in_slab * 128
                    sub_in_slab += 1
                    lt = cmbT[:, col : col + 128]
                    nc.tensor.matmul(
                        psG[:, ls // 8, 64 * (ls % 8) : 64 * (ls % 8) + 64],
                        lhsT=lt,
                        rhs=w64_t[:],
                        start=True,
                        stop=True,
                    )
                    nc.tensor.matmul(
                        psD[:, D_H * (s % DSUB) : D_H * (s % DSUB) + D_H],
                        lhsT=lt,
                        rhs=w16_t[:],
                        start=True,
                        stop=True,
                    )
                    if ls == g_len - 1:
                        nbank = (g_len * 64) // 512
                        nc.scalar.activation(
                            gates[:, g_start * 64 : (g_start + g_len) * 64],
                            psG[:, 0:nbank, :].rearrange("p a b -> p (a b)"),
                            AF.Tanh,
                        )
                        psG = None
                    if s % DSUB == DSUB - 1:
                        nc.vector.tensor_copy(
                            zdb[:, (s - DSUB + 1) * D_H : (s + 1) * D_H],
                            psD[:],
                        )
                # cp/dt after the x/h slabs: they're needed only by the
                # chain, and late emission keeps SP's DMA holds short.
                cp_t = cp_pool.tile([128, JH], F16, tag="cp", name=f"cp{g}")
                nc.sync.dma_start(r3(cp_t[:]), cp[:, g0 : g0 + J, :])
                dt_t = dt_pool.tile([128, J], F16, tag="dt", name=f"dt{g}")
                nc.sync.dma_start(dt_t[:], dt[:, g0 : g0 + J])
                # Pre-broadcast dt to [128, J, 16] on Pool (off the critical
                # path) so the chain's u-mult is a contiguous 2x DVE op
                # instead of a slow strided-broadcast mult.
                dtb_t = dtb_pool.tile([128, JH], F16, tag="dtb", name=f"dtb{g}")
                nc.gpsimd.tensor_copy(
                    r3(dtb_t[:]),
                    dt_t[:].unsqueeze(2).broadcast_to((128, J, D_H)),
                )
                state[g] = (cp_t, dtb_t, gates, zdb)

            flush = {}

            def do_flush(g, final=False):
                # Output DMAs for group g are emitted one iteration after
                # chain(g) computed them, so the Pool sequencer never waits
                # on the chain: the data is long since ready.  The final
                # flush has no work to hide under, so split it across the
                # Pool and SP queues to halve the tail.
                g0 = g * J
                hc_t = flush.pop(g)
                hc3 = r3(hc_t[:], inner=2 * D_H)
                n_fl = 8 if final else N_SLICE
                FS = J // n_fl
                for s in range(n_fl):
                    js = slice(s * FS, (s + 1) * FS)
                    eng = nc.sync if (final and s % 2) else nc.gpsimd
                    eng.dma_start(
                        hc[:, g0 + s * FS : g0 + (s + 1) * FS, :],
                        hc3[:, js, :],
                    )

            def chain(g, n_slice=N_SLICE):
                g0 = g * J
                JS = J // n_slice
                cp_t, dtb_t, gates, zdb = state.pop(g)
                if g >= 1:
                    do_flush(g - 1)
                # softplus(zd) = ln(1 + exp(zd)), full-mega ops (one
                # natural_log table window per mega-group)
                nc.scalar.activation(zdb[:], zdb[:], AF.Exp)
                nc.scalar.activation(zdb[:], zdb[:], AF.Ln, bias=1.0)
                hc_t = hc_pool.tile([128, J * 2 * D_H], F16, tag="hc",
                                    name=f"hc{g}")
                g4 = r3(gates[:], inner=64)
                hc3 = r3(hc_t[:], inner=2 * D_H)
                n_act = min(ACT_SLICE, n_slice)
                AS = J // n_act
                act_sl = [
                    slice(a * AS * D_H, (a + 1) * AS * D_H)
                    for a in range(n_act)
                ]
                for s in range(n_slice):
                    fs = slice(s * JS * D_H, (s + 1) * JS * D_H)
                    # u = sp * dt (DVE 2x)
                    nc.vector.tensor_tensor(
                        zdb[:, fs], zdb[:, fs], dtb_t[:, fs], ALU.mult
                    )
                for fs in act_sl:
                    # E = exp(-u) (ACT, shares the tanh table set)
                    nc.scalar.activation(zdb[:, fs], zdb[:, fs], AF.Exp,
                                         scale=-1.0)
                for s in range(n_slice):
                    js = slice(s * JS, (s + 1) * JS)
                    fs = slice(s * JS * D_H, (s + 1) * JS * D_H)
                    zs3 = r3(zdb[:, fs])
                    cps3 = r3(cp_t[:, fs])
                    cs3 = hc3[:, js, D_H : 2 * D_H]
                    # c_tilde*i into the c_next output slot
                    nc.vector.tensor_tensor(
                        cs3, g4[:, js, 0:16], g4[:, js, 48:64], ALU.mult
                    )
                    # f*c_prev, then *E (both in place on cp_t)
                    nc.vector.tensor_tensor(
                        cps3, g4[:, js, 16:32], cps3, ALU.mult
                    )
                    nc.vector.tensor_tensor(cps3, cps3, zs3, ALU.mult)
                    # c_next = f*c_decay + i*c~
                    nc.vector.tensor_tensor(cs3, cs3, cps3, ALU.add)
                hct4 = hc_t[:].rearrange("p (a b) -> p a b", b=2 * D_H)
                for fs in act_sl:
                    js = slice(fs.start // D_H, fs.stop // D_H)
                    # tanh(c_next) -> reuse zdb slice (E is dead)
                    nc.scalar.activation(
                        zdb[:, fs], hct4[:, js, D_H : 2 * D_H], AF.Tanh
                    )
                    nc.vector.tensor_tensor(
                        hc3[:, js, 0:D_H], g4[:, js, 32:48], r3(zdb[:, fs]),
                        ALU.mult,
                    )
                flush[g] = hc_t

            for g in range(n_mega + 1):
                if g < n_mega:
                    phase_a(g)
                if g >= 1:
                    chain(g - 1,
                          n_slice=8 if g - 1 == n_mega - 1 else N_SLICE)
            do_flush(n_mega - 1, final=True)

    nc.compile()
    return nc


def marshal_core_inputs(x, h_prev, c_prev, delta_t, w64_np, w16_np, lo, hi):
    """Build one core's input map from a batch slice [lo, hi)."""
    rows = hi - lo
    nm = rows // 128
    xs = np.ascontiguousarray(x[lo:hi].T.astype(np.float16))
    hs = np.empty((D_H + 1, rows), np.float16)
    hs[:D_H] = h_prev[lo:hi].T
    hs[D_H] = 1.0  # bias row
    # device row (p, jcol) <-> original row jcol*128 + p
    cps = np.ascontiguousarray(
        c_prev[lo:hi].astype(np.float16).reshape(nm, 128, D_H).transpose(1, 0, 2)
    )
    dts = np.ascontiguousarray(delta_t[lo:hi].astype(np.float16).reshape(nm, 128).T)
    return {"xT": xs, "hT": hs, "cp": cps, "dt": dts, "w64": w64_np, "w16": w16_np}


def unmarshal_output(dev_out, rows):
    """[128, nm, 32] packed fp16 -> ([rows,16], [rows,16]) fp32 batch-major."""
    out = np.asarray(dev_out, np.float32).transpose(1, 0, 2).reshape(rows, 2 * D_H)
    return np.ascontiguousarray(out[:, :D_H]), np.ascontiguousarray(out[:, D_H:])


_PROGRAM_CACHE = {}


def _get_program(rows, mega, chunk):
    key = (rows, mega, chunk)
    if key not in _PROGRAM_CACHE:
        _PROGRAM_CACHE[key] = build_program(rows, mega, chunk)
    return _PROGRAM_CACHE[key]


def run(x, h_prev, c_prev, delta_t, w64_np, w16_np, rows_per_core, mega, chunk,
        trace=False):
    nc = _get_program(rows_per_core, mega, chunk)
    n_cores = N_CORES
    in_maps = [
        marshal_core_inputs(
            x, h_prev, c_prev, delta_t, w64_np, w16_np,
            i * rows_per_core, (i + 1) * rows_per_core,
        )
        for i in range(n_cores)
    ]
    res = run_bass_kernel_spmd(nc, in_maps, list(range(n_cores)), trace=trace)
    parts = [unmarshal_output(res.results[i]["hc"], rows_per_core) for i in range(n_cores)]
    h_next = np.concatenate([p[0] for p in parts], axis=0)
    c_next = np.concatenate([p[1] for p in parts], axis=0)
    return (h_next, c_next), res


def make_weights(W_i, b_i, W_f, b_f, W_o, b_o, W_c, b_c, W_d, b_d):
    """[81,64] fp16 gates block + [81,16] fp16 decay block (bias rows last)."""
    W4 = np.concatenate(
        [np.asarray(w, np.float32) for w in (W_i, W_f, W_o, W_c)], axis=1
    )  # [80, 64]
    b4 = np.concatenate([np.asarray(v, np.float32) for v in (b_i, b_f, b_o, b_c)])
    w64_np = np.ascontiguousarray(
        np.vstack([W4, b4[None, :]]).astype(np.float16)
    )  # [81, 64]
    w16_np = np.ascontiguousarray(
        np.vstack([np.asarray(W_d, np.float32),
                   np.asarray(b_d, np.float32)[None, :]]).astype(np.float16)
    )  # [81, 16]
    return w64_np, w16_np


def kernel(x, h_prev, c_prev, delta_t, W_i, b_i, W_f, b_f, W_o, b_o, W_c, b_c, W_d, b_d):
    x = np.asarray(x, np.float32)
    h_prev = np.asarray(h_prev, np.float32)
    c_prev = np.asarray(c_prev, np.float32)
    delta_t = np.asarray(delta_t, np.float32)
    w64_np, w16_np = make_weights(
        W_i, b_i, W_f, b_f, W_o, b_o, W_c, b_c, W_d, b_d
    )
    (h_next, c_next), _ = run(
        x, h_prev, c_prev, delta_t, w64_np, w16_np,
        rows_per_core=R, mega=32768, chunk=2048,
    )
    return (h_next, c_next)



# revision 98
# speedup vs baseline: 3.1703x; 3.1703x over previous
"""CT-LSTM cell kernel for Trainium2, data-parallel over 8 NeuronCores.

Computes, for B=1048576 rows:
    z = [x, h_prev] @ W + b            (W = concat of 5 [80,16] mats -> [80,80])
    i, f, o, c~ = tanh(z[:, 0:64] split); decay = softplus(z[:, 64:80])
    c_next = f * (c_prev * exp(-decay*dt)) + i * c~
    h_next = o * tanh(c_next)

Cost-model time 106.3us vs 150.8us for the previous ACT-bound pipeline
(1.42x); DMA_ENGINES busy is 94.7us, so the schedule sits ~12% off the
fp16 HBM-traffic roofline -- the 'ridge' regime for this problem.

Key moves over the previous version:
  * The decay factor is algebraically collapsed:
        exp(-dt*softplus(zd)) = (1+e^zd)^(-dt) = sigmoid(-zd)^dt
    so the softplus/exp chain (3 ACT passes + 2 activation-table switches
    per group) becomes ONE ACT sigmoid pass reading the decay PSUM banks
    directly (which also kills the old DVE fp32 drain copy) plus ONE
    gpsimd tensor_tensor pow per bank (the Q7 DSPs implement
    AluOpType.pow in software; ~5e-4 rel err verified on hardware, and
    the cost model prices Pool at 0.6 efficiency).  tanh+sigmoid live in
    the same activation table set -> zero table switches.
  * tanh(c_next) mostly leaves ACT too: 3 of 4 quarters per group use a
    gpsimd pow-pair, tanh(c) = 1 - 2/(1+e^2c) via y = (e^2)^c and
    w = (y+1)^-1 (broadcast-base pow), with the +1 / h = -2*(w*o) + o
    steps on DVE (tensor_scalar / scalar_tensor_tensor).  One quarter
    stays on ACT to use its remaining headroom.  This balances
    ACT ~21us / Pool ~20us / DVE ~17us per group under the 23.7us DMA
    window.
  * Queue discipline (this cost model holds an issuing sequencer while a
    DMA's semaphore waits are unmet, and engine wait-queues are 4 deep
    and in-order): SP carries only input DMAs (dt first, then x/h slabs
    with a 3-deep prefetch ring, then c_prev); Pool carries the powE
    banks, the tanh pow-pairs, and ONLY data-ready flush DMAs (outputs
    deferred two groups so they never wait); ACT carries drains +
    sigmoids only.  The final group's chain runs on the ACT route with
    per-quarter inline flushes on SP to shorten the exposed tail.
  * fp16 everywhere off-chip; PSUM gate groups of 24 subtiles (3 banks
    filled exactly, one contiguous 1536-col tanh drain), decay banks of
    32 subtiles (one full bank per sigmoid/pow).
"""

import sys

import numpy as np

sys.path.insert(0, "/opt/trn_rl_repo")

from concourse import bacc, bass, mybir, tile  # noqa: E402
from concourse.bass_utils import run_bass_kernel_spmd  # noqa: E402

F32 = mybir.dt.float32
F16 = mybir.dt.float16
AF = mybir.ActivationFunctionType
ALU = mybir.AluOpType

N_CORES = 8
BATCH = 1048576
R = BATCH // N_CORES  # rows per core = 131072
D_X = 64
D_H = 16
KD = D_X + D_H + 1  # 81 contraction rows (incl. bias row)
import os as _os

DMACHUNK = int(_os.environ.get("K_DMACHUNK", "8192"))  # x/h slab columns
GSUB = int(_os.environ.get("K_GSUB", "24"))  # subtiles per gate psum group
DSUB = 32  # subtiles per decay psum bank (32 x 16 cols = 2KB)
# tanh(c_next) quarters computed on ACT per group (the rest go through the
# gpsimd pow-pair: tanh(c) = 1 - 2/(1+e^2c)).  ACT has ~3.5us/group of
# headroom under the DMA roofline; one quarter on ACT balances best.
TCACT = int(_os.environ.get("K_TCACT", "1"))
E2CONST = 7.38905609893065  # e^2


def build_program(rows, mega, chunk, n_cores=N_CORES):
    """Build + compile the Bass program (same NEFF for every core)."""
    assert rows % 4096 == 0
    # Tapered group sizes (units of 4096 rows): the elementwise chain of
    # group g executes during group g+1's window, so the last windows
    # shrink to keep the final un-overlapped chain (the tail) short.
    units = [int(u) for u in
             _os.environ.get("K_GROUPS", "8,8,8,8").split(",")]
    assert sum(units) * 4096 == rows, (units, rows)
    megas = [u * 4096 for u in units]
    n_mega = len(megas)
    offs = []
    o = 0
    for m in megas:
        offs.append(o // 128)
        o += m
    jcols = rows // 128

    nc = bacc.Bacc(
        "TRN2",
        target_bir_lowering=False,
        debug=False,
        num_devices=n_cores,
    )
    xT = nc.dram_tensor("xT", [D_X, rows], F16, kind="ExternalInput").ap()
    hT = nc.dram_tensor("hT", [D_H + 1, rows], F16, kind="ExternalInput").ap()
    cp = nc.dram_tensor("cp", [128, jcols, D_H], F16, kind="ExternalInput").ap()
    dt = nc.dram_tensor("dt", [128, jcols], F16, kind="ExternalInput").ap()
    w64 = nc.dram_tensor("w64", [KD, 64], F16, kind="ExternalInput").ap()
    w16 = nc.dram_tensor("w16", [KD, D_H], F16, kind="ExternalInput").ap()
    # packed output: [..., 0:16] = h_next, [..., 16:32] = c_next
    hc = nc.dram_tensor("hc", [128, jcols, 2 * D_H], F16, kind="ExternalOutput").ap()

    with tile.TileContext(nc) as tc:
        with (
            tc.tile_pool(name="wbp", bufs=1) as wbp,
            tc.tile_pool(name="cmb", bufs=int(_os.environ.get("K_CMBBUFS", "4"))) as cmb_pool,
            tc.tile_pool(name="psG", bufs=(1 if GSUB == 48 else 2),
                         space="PSUM") as psG_pool,
            tc.tile_pool(name="psD", bufs=2, space="PSUM") as psD_pool,
            tc.tile_pool(name="gates", bufs=2) as gates_pool,
            tc.tile_pool(name="sdb", bufs=2) as sdb_pool,
            tc.tile_pool(name="tcb", bufs=int(_os.environ.get("K_TCBUFS", "4"))) as tcb_pool,
            tc.tile_pool(name="cpt", bufs=2) as cp_pool,
            tc.tile_pool(name="dtt", bufs=2) as dt_pool,
            tc.tile_pool(name="hcout", bufs=int(_os.environ.get("K_HCBUFS", "2"))) as hc_pool,
        ):
            # weights ride the Pool queue so SP can start the first x/h
            # slabs immediately (and the ACT queue stays clear for the
            # activation-table load)
            w64_t = wbp.tile([KD, 64], F16)
            nc.gpsimd.dma_start(w64_t[:], w64[:, :])
            w16_t = wbp.tile([KD, D_H], F16)
            nc.gpsimd.dma_start(w16_t[:], w16[:, :])
            # broadcast bases for the gpsimd tanh pow-pair
            kb_e2 = wbp.tile([128, D_H], F16)
            nc.gpsimd.memset(kb_e2[:], E2CONST)
            kb_m1 = wbp.tile([128, D_H], F16)
            nc.gpsimd.memset(kb_m1[:], -1.0)

            # Software-pipelined emission: phase A (DMA + GEMM + drains) of
            # group g is emitted BEFORE the elementwise chain of group g-1 so
            # the serial chain hides under dense work.
            state = {}

            def r3(ap2d, inner=D_H):
                return ap2d.rearrange("p (a b) -> p a b", b=inner)

            def phase_a(g):
                mega = megas[g]
                J = mega // 128
                JH = J * D_H
                g0 = offs[g]
                dmachunk = min(DMACHUNK, mega)
                # sdb: sigmoid(-zd) per decay bank, overwritten in place by
                # pow(., dt) on gpsimd -> holds E = exp(-decay*dt)
                sdb = sdb_pool.tile([128, JH], F16, tag="sd", name=f"sd{g}")
                sd3 = r3(sdb[:])
                gates = gates_pool.tile([128, J * 64], F16, tag="gt",
                                        name=f"gt{g}")
                dt_t = dt_pool.tile([128, J], F16, tag="dt", name=f"dt{g}")
                cp_t = cp_pool.tile([128, JH], F16, tag="cp", name=f"cp{g}")
                state[g] = {"cp": cp_t, "dt": dt_t, "gates": gates,
                            "sdb": sdb, "hc": None, "J": J, "g0": g0}
                # Deferred flush of g-2 and the whole chain of g-1, emitted
                # BEFORE this window's production: all inputs completed last
                # window, so on the Pool queue the chain's pows dispatch
                # ahead of this window's bank-paced powE instead of behind
                # powE b7 -- h(g-1) lands mid-window and the last window's
                # DVE only carries the final chain.
                for q in range(4):
                    flush_quarter(g - 2, q, eng=nc.gpsimd)
                for a in range(4):
                    chain_quarter(g - 1, a, part="c")
                for a in range(4):
                    chain_quarter(g - 1, a, part="h")

                # PSUM: gate groups of 24 subtiles (3 banks, 8 x 64 cols
                # filling each bank exactly) + decay banks of 32 subtiles
                # (32 x 16 cols = one full bank); 3*2 + 1*2 = 8 banks.
                # The very first slabs are smaller so the pipeline fills
                # sooner (first drain isn't gated on a full 4096-col load).
                if g == 0:
                    r0 = int(_os.environ.get("K_RAMP", "4096"))
                    rem = (mega - 2 * r0) % dmachunk
                    sizes = ([r0, r0] + ([rem] if rem else [])
                             + [dmachunk] * ((mega - 2 * r0) // dmachunk))
                else:
                    sizes = [dmachunk] * (mega // dmachunk)
                slabs = []
                roff = g0 * 128
                for sz in sizes:
                    slabs.append((roff, sz // 128))
                    roff += sz
                si = 0
                sub_in_slab = 0
                n_sub = 0
                cmbT = None
                psG = None
                psD = None
                g_start = 0
                g_len = 0
                for s in range(J):
                    if sub_in_slab == n_sub:
                        off, n_sub = slabs[si]
                        si += 1
                        sub_in_slab = 0
                        cmbT = cmb_pool.tile([KD, n_sub * 128], F16,
                                             name="cmbT")
                        nc.sync.dma_start(
                            cmbT[0:D_X, :], xT[:, off : off + n_sub * 128]
                        )
                        nc.sync.dma_start(
                            cmbT[D_X:KD, :], hT[:, off : off + n_sub * 128]
                        )
                        # dt after the first slab pair (tiny transfer; the
                        # first slab reaches the matmuls ~0.7us sooner, and
                        # the first powE bank only needs dt ~7us in)
                        if si == int(_os.environ.get("K_DTSI", "1")):
                            nc.sync.dma_start(dt_t[:], dt[:, g0 : g0 + J])


                    if psG is None:
                        g_start = s
                        # group 0 ramps with an 8- then 16-subtile drain so
                        # ACT starts ~3us earlier (first drain only needs
                        # the first 1024-row slab)
                        if g == 0 and s == 0:
                            g_len = 8
                        elif g == 0 and s == 8:
                            g_len = 16
                        else:
                            g_len = min(GSUB, J - s)
                        psG = psG_pool.tile(
                            [128, GSUB // 8, 512], F32, name="psG")
                    if s % DSUB == 0:
                        psD = psD_pool.tile([128, 512], F32, name="psD")
                    ls = s - g_start
                    col = sub_in_slab * 128
                    sub_in_slab += 1
                    lt = cmbT[:, col : col + 128]
                    nc.tensor.matmul(
                        psG[:, ls // 8, 64 * (ls % 8) : 64 * (ls % 8) + 64],
                        lhsT=lt,
                        rhs=w64_t[:],
                        start=True,
                        stop=True,
                    )
                    nc.tensor.matmul(
                        psD[:, D_H * (s % DSUB) : D_H * (s % DSUB) + D_H],
                        lhsT=lt,
                        rhs=w16_t[:],
                        start=True,
                        stop=True,
                    )
                    if ls == g_len - 1:
                        nbank = (g_len * 64) // 512
                        nc.scalar.activation(
                            gates[:, g_start * 64 : (g_start + g_len) * 64],
                            psG[:, 0:nbank, :].rearrange("p a b -> p (a b)"),
                            AF.Tanh,
                        )
                        psG = None
                    if s % DSUB == DSUB - 1:
                        # sigmoid(-zd) straight out of PSUM: this IS the
                        # softplus path (E = sigmoid(-zd)^dt, finished by
                        # the inline gpsimd pow below).
                        nc.scalar.activation(
                            sdb[:, (s - DSUB + 1) * D_H : (s + 1) * D_H],
                            psD[:],
                            AF.Sigmoid,
                            scale=-1.0,
                        )
                        # E = sigmoid(-zd)^dt for this bank (dt broadcast
                        # along the 16 hidden cols via a stride-0 AP); Pool
                        # is otherwise idle here.
                        js = slice(s + 1 - DSUB, s + 1)
                        nc.gpsimd.tensor_tensor(
                            sd3[:, js, :],
                            sd3[:, js, :],
                            dt_t[:, js].unsqueeze(2).broadcast_to(
                                (128, DSUB, D_H)),
                            ALU.pow,
                        )
                # c_prev after the x/h slabs: needed only by the chain,
                # issuing it mid-stream would delay the slab transfers.
                nc.sync.dma_start(r3(cp_t[:]), cp[:, g0 : g0 + J, :])
                # last window only: flush(n-2) on SP right after the inputs.
                # Its chain (emitted at this window's start) finishes by
                # ~60% of the window, so SP never holds, and the transfers
                # fill the DMA gap that post-loop Pool emission (queued
                # behind the bank-paced powE b7) would leave.
                if g == n_mega - 1:
                    for q in range(4):
                        flush_quarter(g - 1, q, eng=nc.sync)

            flush = {}

            def flush_quarter(g, q, eng=None):
                # Output DMA for one quarter of group g, emitted two groups
                # later: the data is long since ready, so the queue never
                # waits on it.
                if g < 0 or g not in flush:
                    return
                hc_t, g0, J = flush[g]
                hc3 = r3(hc_t[:], inner=2 * D_H)
                QF = J // 4
                js = slice(q * QF, (q + 1) * QF)
                (eng or nc.gpsimd).dma_start(
                    hc[:, g0 + js.start : g0 + js.stop, :],
                    hc3[:, js, :],
                )
                if q == 3:
                    del flush[g]

            def chain_quarter(g, a, final=False, part="ch", flushq=None):
                """Emit quarter a (J/4 subtiles) of group g's elementwise
                chain: c_next into the packed output tile, then h_next via
                either ACT tanh or the gpsimd pow-pair
                    tanh(c) = 1 - 2/(1 + e^2c).
                Non-final quarters are hosted at phase_a(g+1)'s bank
                boundaries so their Pool/DVE ops interleave with powE(g+1).
                """
                if g < 0:
                    return
                st = state[g]
                J, g0 = st["J"], st["g0"]
                JH = J * D_H
                if st["hc"] is None:
                    st["hc"] = hc_pool.tile([128, J * 2 * D_H], F16,
                                            tag="hc", name=f"hc{g}")
                    flush[g] = (st["hc"], g0, J)
                hc_t = st["hc"]
                gates, sdb, cp_t = st["gates"], st["sdb"], st["cp"]
                g4 = r3(gates[:], inner=64)
                hc3 = r3(hc_t[:], inner=2 * D_H)
                QF = J // 4
                # finer DVE slices on the final group: nothing hides the
                # serial chain there, so shorter ops start (and finish) sooner
                nsub = 2 if final else 1
                SS = QF // nsub
                do1 = part in ("c", "ch", "c1")
                do2 = part in ("c", "ch", "c2")
                for t in range(nsub if (do1 or do2) else 0):
                    js = slice(a * QF + t * SS, a * QF + (t + 1) * SS)
                    fs = slice(js.start * D_H, js.stop * D_H)
                    es3 = r3(sdb[:, fs])
                    cps3 = r3(cp_t[:, fs])
                    cs3 = hc3[:, js, D_H : 2 * D_H]
                    if do1:
                        # c_tilde*i into the c_next output slot -- the only
                        # c-op with no c_prev dependency, emitted first so
                        # DVE can run it before the cp DMA lands
                        nc.vector.tensor_tensor(
                            cs3, g4[:, js, 0:16], g4[:, js, 48:64], ALU.mult
                        )
                    if do2:
                        # f*c_prev, then *E (both in place on cp_t)
                        nc.vector.tensor_tensor(
                            cps3, g4[:, js, 16:32], cps3, ALU.mult
                        )
                        nc.vector.tensor_tensor(cps3, cps3, es3, ALU.mult)
                        # c_next = f*c_decay + i*c~
                        nc.vector.tensor_tensor(cs3, cs3, cps3, ALU.add)
                if "h" not in part:
                    return
                tcb = tcb_pool.tile([128, JH // 4], F16, tag="tc",
                                    name=f"tc{g}_{a}")
                js = slice(a * QF, (a + 1) * QF)
                fs = slice(js.start * D_H, js.stop * D_H)
                h3 = hc3[:, js, 0:D_H]
                o3 = g4[:, js, 32:48]
                c3 = hc3[:, js, D_H : 2 * D_H]
                # First TCACT quarters on ACT (it has headroom under the
                # DMA roofline); the rest via the gpsimd pow-pair.  The
                # final group's LAST quarter also goes on ACT: it is past
                # the last drain, ACT is free, and the ACT route has the
                # shorter serial latency for the exposed tail.
                if final or a < TCACT:
                    nc.scalar.activation(tcb[:], c3, AF.Tanh)
                    nc.vector.tensor_tensor(h3, o3, r3(tcb[:]), ALU.mult)
                else:
                    y3 = r3(tcb[:])
                    # y = e^(2c)
                    nc.gpsimd.tensor_tensor(
                        y3,
                        kb_e2[:].unsqueeze(1).broadcast_to((128, QF, D_H)),
                        c3,
                        ALU.pow,
                    )
                    # u = y + 1
                    nc.vector.tensor_scalar(y3, y3, 1.0, None, ALU.add)
                    # w = 1/u
                    nc.gpsimd.tensor_tensor(
                        y3,
                        y3,
                        kb_m1[:].unsqueeze(1).broadcast_to((128, QF, D_H)),
                        ALU.pow,
                    )
                    # h = o*(1 - 2w) = -2*(w*o) + o
                    nc.vector.tensor_tensor(h3, y3, o3, ALU.mult)
                    nc.vector.scalar_tensor_tensor(
                        h3, h3, -2.0, o3, ALU.mult, ALU.add
                    )
                if final if flushq is None else flushq:
                    # nothing runs after the last chain: stream each finished
                    # quarter out immediately so the transfer overlaps the
                    # remaining quarters.  Always on SP: a data-dependent DMA
                    # on the Pool queue would hold the Pool SEQ and trickle
                    # every queued pow through the 4-deep wait queue.
                    nc.sync.dma_start(
                        hc[:, g0 + js.start : g0 + js.stop, :],
                        hc3[:, js, :],
                    )
                if a == 3:
                    state.pop(g)

            for g in range(n_mega):
                phase_a(g)
            # post-loop drain: the final chain via the ACT route (ACT is
            # idle past the last drain) with inline flushes; flush(n-2)
            # already rode SP at the end of the last window.
            # final chain in FSEG fine segments: each [c-ops, ACT tanh,
            # h-mult, inline SP flush] so the exposed tail is only ONE
            # segment's serial latency past the last powE bank.
            FSEG = int(_os.environ.get("K_FSEG", "8"))
            gl = n_mega - 1
            stl = state.pop(gl)
            Jl, gl0 = stl["J"], stl["g0"]
            hc_l = hc_pool.tile([128, Jl * 2 * D_H], F16, tag="hc",
                                name=f"hc{gl}")
            g4l = r3(stl["gates"][:], inner=64)
            hc3l = r3(hc_l[:], inner=2 * D_H)
            SG = Jl // FSEG
            for a in range(FSEG):
                js = slice(a * SG, (a + 1) * SG)
                fs = slice(js.start * D_H, js.stop * D_H)
                es3 = r3(stl["sdb"][:, fs])
                cps3 = r3(stl["cp"][:, fs])
                cs3 = hc3l[:, js, D_H : 2 * D_H]
                nc.vector.tensor_tensor(
                    cs3, g4l[:, js, 0:16], g4l[:, js, 48:64], ALU.mult)
                nc.vector.tensor_tensor(
                    cps3, g4l[:, js, 16:32], cps3, ALU.mult)
                nc.vector.tensor_tensor(cps3, cps3, es3, ALU.mult)
                nc.vector.tensor_tensor(cs3, cs3, cps3, ALU.add)
                tcb = tcb_pool.tile([128, SG * D_H], F16, tag="tc",
                                    name=f"tcf{a}")
                nc.scalar.activation(tcb[:], cs3, AF.Tanh)
                nc.vector.tensor_tensor(
                    hc3l[:, js, 0:D_H], g4l[:, js, 32:48], r3(tcb[:]),
                    ALU.mult)
                nc.sync.dma_start(
                    hc[:, gl0 + js.start : gl0 + js.stop, :],
                    hc3l[:, js, :])

    nc.compile()
    return nc


def marshal_core_inputs(x, h_prev, c_prev, delta_t, w64_np, w16_np, lo, hi):
    """Build one core's input map from a batch slice [lo, hi)."""
    rows = hi - lo
    nm = rows // 128
    xs = np.ascontiguousarray(x[lo:hi].T.astype(np.float16))
    hs = np.empty((D_H + 1, rows), np.float16)
    hs[:D_H] = h_prev[lo:hi].T
    hs[D_H] = 1.0  # bias row
    # device row (p, jcol) <-> original row jcol*128 + p
    cps = np.ascontiguousarray(
        c_prev[lo:hi].astype(np.float16).reshape(nm, 128, D_H).transpose(1, 0, 2)
    )
    dts = np.ascontiguousarray(delta_t[lo:hi].astype(np.float16).reshape(nm, 128).T)
    return {"xT": xs, "hT": hs, "cp": cps, "dt": dts, "w64": w64_np, "w16": w16_np}


def unmarshal_output(dev_out, rows):
    """[128, nm, 32] packed fp16 -> ([rows,16], [rows,16]) fp32 batch-major."""
    out = np.asarray(dev_out, np.float32).transpose(1, 0, 2).reshape(rows, 2 * D_H)
    return np.ascontiguousarray(out[:, :D_H]), np.ascontiguousarray(out[:, D_H:])


_PROGRAM_CACHE = {}


def _get_program(rows, mega, chunk):
    key = (rows, mega, chunk)
    if key not in _PROGRAM_CACHE:
        _PROGRAM_CACHE[key] = build_program(rows, mega, chunk)
    return _PROGRAM_CACHE[key]


def run(x, h_prev, c_prev, delta_t, w64_np, w16_np, rows_per_core, mega, chunk,
        trace=False):
    nc = _get_program(rows_per_core, mega, chunk)
    n_cores = N_CORES
    in_maps = [
        marshal_core_inputs(
            x, h_prev, c_prev, delta_t, w64_np, w16_np,
            i * rows_per_core, (i + 1) * rows_per_core,
        )
        for i in range(n_cores)
    ]
    res = run_bass_kernel_spmd(nc, in_maps, list(range(n_cores)), trace=trace)
    parts = [unmarshal_output(res.results[i]["hc"], rows_per_core) for i in range(n_cores)]
    h_next = np.concatenate([p[0] for p in parts], axis=0)
    c_next = np.concatenate([p[1] for p in parts], axis=0)
    return (h_next, c_next), res


def make_weights(W_i, b_i, W_f, b_f, W_o, b_o, W_c, b_c, W_d, b_d):
    """[81,64] fp16 gates block + [81,16] fp16 decay block (bias rows last)."""
    W4 = np.concatenate(
        [np.asarray(w, np.float32) for w in (W_i, W_f, W_o, W_c)], axis=1
    )  # [80, 64]
    b4 = np.concatenate([np.asarray(v, np.float32) for v in (b_i, b_f, b_o, b_c)])
    w64_np = np.ascontiguousarray(
        np.vstack([W4, b4[None, :]]).astype(np.float16)
    )  # [81, 64]
    w16_np = np.ascontiguousarray(
        np.vstack([np.asarray(W_d, np.float32),
                   np.asarray(b_d, np.float32)[None, :]]).astype(np.float16)
    )  # [81, 16]
    return w64_np, w16_np


def kernel(x, h_prev, c_prev, delta_t, W_i, b_i, W_f, b_f, W_o, b_o, W_c, b_c, W_d, b_d):
    x = np.asarray(x, np.float32)
    h_prev = np.asarray(h_prev, np.float32)
    c_prev = np.asarray(c_prev, np.float32)
    delta_t = np.asarray(delta_t, np.float32)
    w64_np, w16_np = make_weights(
        W_i, b_i, W_f, b_f, W_o, b_o, W_c, b_c, W_d, b_d
    )
    (h_next, c_next), _ = run(
        x, h_prev, c_prev, delta_t, w64_np, w16_np,
        rows_per_core=R, mega=32768, chunk=2048,
    )
    return (h_next, c_next)
